# revision 55
# baseline (speedup 1.0000x reference)
"""MoE (E=8 experts, top-2, D=1024, T=8192) — expert-parallel Trainium2 kernel.

Strategy (per the expert-parallel sharding hint):
  - Host computes the gate (0.1% of FLOPs: scores, top-2, softmax) and uses it
    to shard tokens: each of the 8 NeuronCores owns one expert and receives
    exactly the tokens routed to it (padded to a common capacity C).
  - Each core runs the dense expert FFN + layernorm + combine-weight scaling
    over its routed tokens: 99.9% of the FLOPs.
  - Host gathers the per-expert outputs back into token order (pure gather —
    slot assignment makes a scatter unnecessary) and sums the K=2 contributions.

Device dataflow — mm1 weights-stationary, mm2 TOKENS-stationary:
  mm1: stationary w1 chunks, moving xT [d, tok] -> psum hT [2D-feat, tok]
       -> gelu -> hT fp8 (features on partitions).
  mm2: stationary hT chunks [f=256(DR), tok=128], moving w2 [f, d] -> psum
       z [tok 128, d 512] x2 banks — TOKENS ON PARTITIONS.
  With tokens on partitions the whole LN epilogue runs on free-axis
  primitives and the PE does ZERO stats work (the previous design burned
  ~20% of PE time on ones@z / ones@z^2 stats matmuls):
    z = psum + xres            (DVE tensor_tensor, f32)
    mean/var                   (DVE bn_stats / bn_aggr, exact f32)
    rstd = rsqrt(var+eps)      (DVE-only bit-trick seed + 2 Newton steps —
                                keeps Sqrt off ACT so the gelu activation
                                table is never thrashed)
    out = (z - mu)*rstd*wv     (ONE fused DVE tensor_scalar per 512-bank,
                                per-partition scalar operands)
  and the output DMA writes token-major [C, D] rows (fat 4KB descriptors,
  pure row-gather combine on host).

The FFN matmuls run in fp8-e4m3 with perf_mode=DoubleRow (measured peak:
512-col DR matmul = 215ns = 157 TF/s). Weights are host-prescaled by 2^10
(and x by 2^4 for mm1) so the e4m3 lattice is used well clear of denormals;
the gelu activation unscales mm1 by 2^-14. mm2's 2^10 output scale is never
unscaled on the z path: layernorm is scale-invariant (eps is scaled by 2^20,
rstd comes out 2^-10-scaled and cancels it), so the device output is the
exact unscaled wv-weighted LN. w1/w2 are fully SBUF-resident in fp8 (4 MB),
host-relaid so every DMA lands as one fat contiguous descriptor per
partition. mm1 is software-pipelined one tile ahead of mm2, with mm2
accumulation groups interleaved between mm1 groups so the in-order PE always
has an independent group queued behind any group waiting on DMA or the DVE.
"""

import sys

sys.path.insert(0, "/opt/trn_rl_repo")

import ml_dtypes
import numpy as np

E, K, D = 8, 2, 1024
H = 2 * D  # ffn hidden
B, S = 2, 4096
T = B * S
CT = 512  # max token tile (PSUM bank = 512 f32)
CT_MIN = 32  # remainder-tile granularity
P = 128
NCH = CT // P  # 4 token chunks per tile
KO1 = D // P  # 8  k-chunks for mm1 (4 DoubleRow pairs)
MO1 = H // P  # 16 m-chunks for mm1
KO2 = H // P  # 16 k-chunks for mm2 (8 DoubleRow pairs)

SW = 1024.0  # weight prescale (2^10)
SX = 16.0  # x prescale for the fp8 mm1 moving operand (2^4)
FP8_MAX = 240.0  # TRN float8e4 max normal
MAGICF = float(0x5F3759DF)  # rsqrt seed magic (bit trick in float arithmetic)

_kernel_cache = {}


def _tiles_for(C):
    tiles = [(i * CT, CT) for i in range(C // CT)]
    if C % CT:
        tiles.append((C - C % CT, C % CT))
    return tiles


def _cws_for(ct):
    """Token-chunk widths of a tile: e.g. 512 -> [128,128,128,128]; 160 -> [128,32]."""
    return [min(P, ct - c * P) for c in range((ct + P - 1) // P)]


def _build_bass(C, use_lng, use_lnb):
    """Build the per-core expert-FFN kernel for capacity C (multiple of CT_MIN)."""
    import concourse.tile as tile
    from concourse import bacc, mybir

    f32 = mybir.dt.float32
    i32 = mybir.dt.int32
    fp8 = mybir.dt.float8e4
    AF = mybir.ActivationFunctionType
    OP = mybir.AluOpType
    DR = mybir.MatmulPerfMode.DoubleRow

    assert C % CT_MIN == 0
    tiles = _tiles_for(C)
    NT = len(tiles)

    nc = bacc.Bacc("TRN2", target_bir_lowering=False, debug=False)
    # xT/xres ship in per-tile-padded block layouts so each tile's DMA is one
    # contiguous descriptor per partition
    xT_d = nc.dram_tensor("xT", [P, NT, KO1, CT], fp8, kind="ExternalInput")
    xres_d = nc.dram_tensor("xres", [P, NT, NCH, D], f32, kind="ExternalInput")
    w1_d = nc.dram_tensor("w1", [P, MO1, KO1, P], fp8, kind="ExternalInput")
    # w2 moving layout, bank-major: [p, bank, k, j, d'] = w2[(2k+j)*128+p,
    # bank*512+d'] (DR pair on j). Bank-major lets mm2 run all bank-0 groups
    # before any bank-1 group, so the startup only races the FIRST 1MB of w2.
    w2_d = nc.dram_tensor("w2", [P, 2, KO2 // 2, 2, 512], fp8, kind="ExternalInput")
    b1_d = nc.dram_tensor("b1", [P, MO1], f32, kind="ExternalInput")
    wv_d = nc.dram_tensor("wv", [P, NT * NCH], f32, kind="ExternalInput")
    if use_lng:
        lng_d = nc.dram_tensor("ln_g", [P, D], f32, kind="ExternalInput")
    if use_lnb:
        lnb_d = nc.dram_tensor("ln_b", [P, D], f32, kind="ExternalInput")
    bf16 = mybir.dt.bfloat16
    out_d = nc.dram_tensor("out", [C, D], bf16, kind="ExternalOutput")

    with tile.TileContext(nc) as tc:
        with (
            tc.tile_pool(name="singles", bufs=1) as singles,
            tc.tile_pool(name="xp", bufs=3) as xp,
            tc.tile_pool(name="hp", bufs=2) as hp,
            tc.tile_pool(name="zp", bufs=2) as zp,
            tc.tile_pool(name="xrp", bufs=2) as xrp,
            tc.tile_pool(name="stp", bufs=2) as stp,
            tc.tile_pool(name="psmm", bufs=5, space="PSUM") as psmm,
            tc.tile_pool(name="psm2", bufs=3, space="PSUM") as psm2,
        ):
            # ---- resident data ----
            # Startup is HWDGE-ramp-bound (~160 GB/s for the first few us), so
            # the first matmul's dependency set is kept minimal: tile-0 xT is
            # DMAd in two k-halves (one per ring) and w1 chunk 0 in two
            # m-halves, so subtile dep-tracking releases mm1 group 0 after
            # ~384KB instead of ~2.5MB. b1/wv are tiny and go first. The
            # warm-up gelu pulls the ACT_TABLE_LOAD (1.3us) into the DMA wait
            # instead of stalling the first real gelu (and the psum ring).
            # Every dma_start costs ~630ns of ISSUE time on its sequencer and
            # all HWDGE traffic fans out to the same 16 DMA engines, so ring
            # choice is about sequencer headroom, not bandwidth: the whole
            # startup weight stream issues from the (otherwise idle) SP
            # queue, keeping the ACT queue free to start gelu 0 the moment
            # mm1 group 0 closes.
            warm = singles.tile([P, 1], f32)
            nc.vector.memset(warm[:], 0.0)
            nc.scalar.activation(out=warm[:], in_=warm[:], func=AF.Gelu, bias=0.0, scale=0.0)
            xT_tiles = [xp.tile([P, KO1, CT], fp8, name="xT_sb") for _ in range(3)]
            b1_sb = singles.tile([P, MO1], f32)
            wv_sb = singles.tile([P, NT * NCH], f32)
            # w1 lives in 4 tiles grouped by consumption time: m0/m1 as two
            # tiny transfers (first-matmul deps), then three bulk transfers.
            # Each DMA targets a whole tile, so dependency release needs no
            # subtile tracking, and the issue count (5 + xT halves) keeps
            # both sequencers' ~630ns/issue overhead off the critical path.
            w1_groups = [[0, 1], [2, 3], [4, 5], [6, 7, 8, 9], [10, 11, 12, 13, 14, 15]]
            w1_tiles = [
                singles.tile([P, len(ms), KO1, P], fp8, name=f"w1g{i}")
                for i, ms in enumerate(w1_groups)
            ]
            w1_map = {}
            for gi, ms in enumerate(w1_groups):
                for li, m in enumerate(ms):
                    w1_map[m] = (gi, li)
            nc.sync.dma_start(xT_tiles[0][:, 0 : KO1 // 2], xT_d[:, 0, 0 : KO1 // 2])
            nc.scalar.dma_start(b1_sb[:], b1_d[:])
            nc.sync.dma_start(w1_tiles[0][:, 0:1], w1_d[:, 0:1])
            nc.scalar.dma_start(xT_tiles[0][:, KO1 // 2 : KO1], xT_d[:, 0, KO1 // 2 : KO1])
            nc.scalar.dma_start(w1_tiles[0][:, 1:2], w1_d[:, 1:2])
            nc.sync.dma_start(w1_tiles[1][:], w1_d[:, 2:4])
            nc.scalar.dma_start(w1_tiles[2][:], w1_d[:, 4:6])
            nc.scalar.dma_start(w1_tiles[3][:], w1_d[:, 6:10])
            nc.sync.dma_start(w1_tiles[4][:], w1_d[:, 10:16])
            nc.scalar.dma_start(wv_sb[:], wv_d[:])
            # PE warm-up: dep-free dummy matmuls run during the startup DMA
            # wait, ramping the tensor engine's DVFS clock to max so the
            # first real matmuls start at full speed instead of re-ramping
            wsrc = singles.tile([P, 2, 512], fp8)
            nc.gpsimd.memset(wsrc[:], 1.0)
            wps = psmm.tile([P, CT], f32, name="ps_mm")
            for _ in range(20):
                nc.tensor.matmul(
                    wps[:],
                    wsrc[:, :, :128],
                    wsrc[:],
                    start=True,
                    stop=True,
                    perf_mode=DR,
                )
            if use_lng:
                lng_sb = singles.tile([P, D], f32)
                nc.scalar.dma_start(lng_sb[:], lng_d[:])
            if use_lnb:
                lnb_sb = singles.tile([P, D], f32)
                nc.scalar.dma_start(lnb_sb[:], lnb_d[:])
            w2_ch = [singles.tile([P, KO2 // 2, 2, 512], fp8, name=f"w2c{i}") for i in range(2)]

            def emit_w2_chunk(i):
                nc.scalar.dma_start(w2_ch[i][:], w2_d[:, i])

            def new_state(t, t0, ct):
                """Allocated ahead of mm1(t); the xres DMA is emitted inside
                emit_mm1(t) behind a few of tile t's gelus, so it lands a full
                tile before mm2(t) consumes it but never steals HBM bandwidth
                from the startup weight stream."""
                return {
                    "t": t, "t0": t0, "ct": ct, "cws": _cws_for(ct),
                    "hT_sb": None, "norm_prev": None,
                    "xres": xrp.tile([P, NCH, D], f32, name="xres"),
                    "z": zp.tile([P, NCH, 2, 512], f32, name="z_sb"),
                    "occ": zp.tile([P, NCH, D], bf16, name="occ"),
                    "bst": stp.tile([P, NCH, 2, 6], f32, name="bst"),
                    "mv": stp.tile([P, NCH * 2], f32, name="mv"),
                    "rw": stp.tile([P, NCH], f32, name="rw"),
                    "nb": stp.tile([P, NCH], f32, name="nb"),
                }

            def emit_mm1(t, st, prev_st):
                """mm1 (fp8 DoubleRow) + gelu into a fresh hT tile; mm2 groups
                of the previous tile interleave so the PE always has an
                independent accumulation group queued behind a stalled one."""
                ct = st["ct"]
                xT_sb = xT_tiles[t % 3]
                hT_sb = hp.tile([P, KO2, CT], fp8, name="hT_sb")
                for m in range(MO1):
                    gi, li = w1_map[m]
                    ps = psmm.tile([P, CT], f32, name="ps_mm")
                    for k in range(KO1 // 2):
                        nc.tensor.matmul(
                            ps[:, :ct],
                            w1_tiles[gi][:, li, 2 * k : 2 * k + 2, :],
                            xT_sb[:, 2 * k : 2 * k + 2, :ct],
                            start=(k == 0),
                            stop=(k == KO1 // 2 - 1),
                            perf_mode=DR,
                        )
                    # gelu(2^-14 * ps + b1) -> fp8 h (unscaled)
                    nc.scalar.activation(
                        out=hT_sb[:, m, :ct],
                        in_=ps[:, :ct],
                        func=AF.Gelu,
                        bias=b1_sb[:, m : m + 1],
                        scale=1.0 / (SW * SX),
                    )
                    if t == 0 and m in (3, 6):
                        emit_w2_chunk((m - 3) // 3)
                    if t == 0 and m == 4 and NT > 1:
                        # tile-1 xT prefetch goes out BEFORE tile-0's xres
                        # below so the sync ring serves it first
                        nc.sync.dma_start(xT_tiles[1][:], xT_d[:, 1])
                    if m == (6 if t == 0 else 2):
                        # tile 0's xres rides the (lighter) sync ring so the
                        # ACT ring can deliver w2 bank 1 ~3us earlier
                        eng = nc.sync if t == 0 else nc.scalar
                        eng.dma_start(st["xres"][:], xres_d[:, t])
                    # interleave offset 3 (not 1): tile 0's bank-0 z-adds
                    # otherwise outrun the xres(0) DMA, pinning psum banks
                    # and stalling the PE ~1.5us into window 1
                    if prev_st is not None and 3 <= m <= 10:
                        mm2_group(prev_st, m - 3)
                st["hT_sb"] = hT_sb

            def mm2_group(st, slot, cb=None):
                """One (token-chunk, d-bank) accumulation group: psum[tok, d]
                += hT.T @ w2 over all 16 k-chunks, then the z/stats chain.
                The previous tile's (ACT-engine) normalize rides one chunk at
                a time between this tile's gelus. Main-loop order is
                bank-major (all bank-0 groups first) so the first half of w2
                isn't needed until 4 groups later than the first."""
                if st["norm_prev"] is not None:
                    norm_chunk(st["norm_prev"], slot)
                b, c = divmod(slot, NCH) if cb is None else cb
                if c >= len(st["cws"]):
                    return
                cw = st["cws"][c]
                ps = psm2.tile([P, 512], f32, name="ps2")
                for k in range(KO2 // 2):
                    nc.tensor.matmul(
                        ps[:cw, :],
                        st["hT_sb"][:, 2 * k : 2 * k + 2, c * P : c * P + cw],
                        w2_ch[b][:, k],
                        start=(k == 0),
                        stop=(k == KO2 // 2 - 1),
                        perf_mode=DR,
                    )
                zc = st["z"][:cw, c, b]
                nc.vector.tensor_tensor(
                    zc, ps[:cw, :], st["xres"][:cw, c, b * 512 : (b + 1) * 512], OP.add
                )
                nc.vector.bn_stats(st["bst"][:cw, c, b], zc)
                if b == 1:
                    nc.vector.bn_aggr(st["mv"][:cw, 2 * c : 2 * c + 2], st["bst"][:cw, c])

            def stats_range(st, lo, hi, newton=2):
                """Batched rstd for chunks [lo, hi): rw[:, c] = wv * rsqrt(
                var_c + eps), computed entirely on the DVE (bit-trick seed +
                2 Newton steps) so the ACT gelu table is never thrashed by a
                Sqrt."""
                t = st["t"]
                mv, rw = st["mv"], st["rw"]
                vb = stp.tile([P, NCH], f32, name="vb")
                # var is 2^20-scaled; eps scales to match and rstd comes out
                # 2^-10-scaled, cancelling z's 2^10 scale in the normalize
                nc.vector.tensor_scalar(
                    vb[:, lo:hi], mv[:, 2 * lo + 1 : 2 * hi : 2], 1.0, 1e-6 * SW * SW,
                    OP.mult, OP.add,
                )
                sd = stp.tile([P, NCH], f32, name="sd")
                nc.vector.tensor_copy(sd[:, lo:hi], vb[:, lo:hi].bitcast(i32))
                nc.vector.tensor_scalar(sd[:, lo:hi], sd[:, lo:hi], -0.5, MAGICF, OP.mult, OP.add)
                y0 = stp.tile([P, NCH], i32, name="y0")
                nc.vector.tensor_copy(y0[:, lo:hi], sd[:, lo:hi])
                yf = y0[:, lo:hi].bitcast(f32)
                a_sb = stp.tile([P, NCH], f32, name="a_sb")
                for _ in range(newton):
                    nc.vector.tensor_tensor(a_sb[:, lo:hi], vb[:, lo:hi], yf, OP.mult)
                    nc.vector.tensor_tensor(a_sb[:, lo:hi], a_sb[:, lo:hi], yf, OP.mult)
                    nc.vector.tensor_scalar(a_sb[:, lo:hi], a_sb[:, lo:hi], -0.5, 1.5, OP.mult, OP.add)
                    nc.vector.tensor_tensor(yf, yf, a_sb[:, lo:hi], OP.mult)
                nc.vector.tensor_tensor(
                    rw[:, lo:hi], yf, wv_sb[:, t * NCH + lo : t * NCH + hi], OP.mult
                )
                # normalize runs on ACT as out = Identity(z*rw + nb) with
                # per-partition scalars, so nb = -mu*rw
                nb = st["nb"]
                nc.vector.tensor_tensor(
                    nb[:, lo:hi], mv[:, 2 * lo : 2 * hi : 2], rw[:, lo:hi], OP.mult
                )
                nc.vector.tensor_scalar(nb[:, lo:hi], nb[:, lo:hi], -1.0, None, OP.mult)

            def norm_chunk(st, g, dve=False):
                """occ = Identity(z*rw - mu*rw) on the ACT engine (Identity
                lives in the gelu table: no table reload, and the DVE stays
                free for the z/bn chain); bank 1 also DMAs the finished
                [cw, D] token rows out (fat row descriptors). dve=True runs
                the same affine on the DVE instead — used for the final
                chunk so its two banks normalize on two engines in parallel."""
                c, b = divmod(g, 2)
                if c >= len(st["cws"]):
                    return
                cw = st["cws"][c]
                zc = st["z"][:cw, c, b]
                oc = st["occ"][:cw, c, b * 512 : (b + 1) * 512]
                if dve:
                    nc.vector.tensor_scalar(
                        oc, zc, st["mv"][:cw, 2 * c : 2 * c + 1],
                        st["rw"][:cw, c : c + 1], OP.subtract, OP.mult,
                    )
                else:
                    nc.scalar.activation(
                        out=oc, in_=zc, func=AF.Identity,
                        bias=st["nb"][:cw, c : c + 1], scale=st["rw"][:cw, c : c + 1],
                    )
                if use_lng:
                    nc.vector.tensor_tensor(
                        oc, oc, lng_sb[:cw, b * 512 : (b + 1) * 512], OP.mult
                    )
                if use_lnb:
                    lb = stp.tile([P, 512], f32, name="lb")
                    nc.vector.tensor_scalar(
                        lb[:cw, :], lnb_sb[:cw, b * 512 : (b + 1) * 512],
                        wv_sb[:cw, st["t"] * NCH + c : st["t"] * NCH + c + 1],
                        None, OP.mult,
                    )
                    nc.vector.tensor_tensor(oc, oc, lb[:cw, :], OP.add)
                if b == 1:
                    r0 = st["t0"] + c * P
                    nc.sync.dma_start(out_d[r0 : r0 + cw, :], st["occ"][:cw, c])

            # xT(t) is consumed only by mm1(t) (residual uses the xres
            # stream); prefetches are queued after the w1 chunks so they never
            # delay the critical w1 stream. stats+normalize+out-DMA of tile t
            # are emitted in window t+1 (right behind mm2(t)'s z chain), so
            # the out DMAs spread across the kernel instead of bunching in
            # the tail; the DVE has ~12us/window of slack for them.
            prev_st = None
            for t, (t0, ct) in enumerate(tiles):
                st = new_state(t, t0, ct)
                emit_mm1(t, st, prev_st)
                for tn_i in ([2] if t == 0 else [t + 2]):
                    if tn_i < NT:
                        nc.sync.dma_start(xT_tiles[tn_i % 3][:], xT_d[:, tn_i])
                if prev_st is not None:
                    stats_range(prev_st, 0, len(prev_st["cws"]))
                    st["norm_prev"] = prev_st
                prev_st = st
            # epilogue: the last tile drains chunk-at-a-time (mm2 -> stats ->
            # norm -> DMA) so each chunk's serial stats/norm chain overlaps
            # the next chunk's matmuls and the tail is one chunk deep; norm
            # of tile NT-2 rides the epilogue mm2 groups as usual
            nch_l = len(prev_st["cws"])
            # a short last tile has fewer mm2 groups than the previous tile
            # has norm chunks — flush the rides the loop below won't reach
            if prev_st["norm_prev"] is not None:
                for g in range(2 * nch_l, 2 * NCH):
                    norm_chunk(prev_st["norm_prev"], g)
            for c in range(nch_l):
                mm2_group(prev_st, 2 * c, cb=(0, c))
                mm2_group(prev_st, 2 * c + 1, cb=(1, c))
                # the final chunk's serial stats chain IS the kernel tail;
                # it is ~all padding slots (only the max-load expert has a
                # handful of real tokens there), so one Newton step suffices
                last = c == nch_l - 1
                stats_range(prev_st, c, c + 1, newton=1 if last else 2)
                norm_chunk(prev_st, 2 * c, dve=last)
                norm_chunk(prev_st, 2 * c + 1)

    nc.finalize()
    return nc


def _route(x, gate_w):
    """Host gate: top-2 per token + softmax combine weights (matches
    jax.lax.top_k tie-breaking: lower index wins)."""
    xt = x.reshape(-1, D)
    scores = xt.astype(np.float32) @ gate_w.astype(np.float32)  # [T, E]
    e0 = np.argmax(scores, axis=1)
    s0 = scores[np.arange(T), e0]
    masked = scores.copy()
    masked[np.arange(T), e0] = -np.inf
    e1 = np.argmax(masked, axis=1)
    s1 = masked[np.arange(T), e1]
    # softmax over the two selected scores
    mx = np.maximum(s0, s1)
    z0 = np.exp((s0 - mx).astype(np.float64))
    z1 = np.exp((s1 - mx).astype(np.float64))
    den = z0 + z1
    w0 = (z0 / den).astype(np.float32)
    w1 = (z1 / den).astype(np.float32)
    return xt, e0, e1, w0, w1


def _fp8(a, scale):
    return np.clip(a * scale, -FP8_MAX, FP8_MAX).astype(ml_dtypes.float8_e4m3)


def _wlay(w, ko, mo):
    """[ko*P, mo*P] -> [P, mo, ko, P]: partition-contiguous DMA layout that
    matches the lhsT access pattern w_sb[:, m, 2k:2k+2, :]."""
    return np.ascontiguousarray(
        np.asarray(w).reshape(ko, P, mo, P).transpose(1, 2, 0, 3)
    )


def kernel(x, gate_w, w1, b1, w2, b2, ln_g, ln_b):
    from concourse.bass_utils import run_bass_kernel_spmd

    x = np.asarray(x)
    xt, e0, e1, wk0, wk1 = _route(x, np.asarray(gate_w))

    # slot assignment: expert e's token list = tokens with e0==e, then e1==e
    idx_e, wv_e = [], []
    for e in range(E):
        i0 = np.nonzero(e0 == e)[0]
        i1 = np.nonzero(e1 == e)[0]
        idx_e.append(np.concatenate([i0, i1]))
        wv_e.append(np.concatenate([wk0[i0], wk1[i1]]))
    maxn = max(len(i) for i in idx_e)
    C = max(CT_MIN, -(-maxn // CT_MIN) * CT_MIN)
    tiles = _tiles_for(C)
    NT = len(tiles)

    b2 = np.asarray(b2, np.float32)
    use_lng = bool(np.any(np.asarray(ln_g) != 1))
    use_lnb = bool(np.any(np.asarray(ln_b) != 0))
    key = (C, use_lng, use_lnb)
    if key not in _kernel_cache:
        _kernel_cache[key] = _build_bass(C, use_lng, use_lnb)
    nc = _kernel_cache[key]

    in_maps = []
    for e in range(E):
        n = len(idx_e[e])
        xTe = np.zeros((D, C), np.float32)
        xTe[:, :n] = xt[idx_e[e]].T
        # mm1 moving operand: per-tile padded fp8 block layout [P, NT, KO1, CT]
        x8 = _fp8(xTe, SX).reshape(KO1, P, C)
        xT_blk = np.zeros((P, NT, KO1, CT), ml_dtypes.float8_e4m3)
        # residual stream, token-major rows (b2 folded in), 2^10-scaled
        xr_full = np.zeros((C, D), np.float32)
        xr_full[:n] = (xt[idx_e[e]] + b2[e]) * SW
        wv_full = np.zeros(C, np.float32)
        wv_full[:n] = wv_e[e]
        xr_blk = np.zeros((P, NT, NCH, D), np.float32)
        wv_blk = np.zeros((P, NT * NCH), np.float32)
        for t, (t0, ct) in enumerate(tiles):
            xT_blk[:, t, :, :ct] = x8[:, :, t0 : t0 + ct].transpose(1, 0, 2)
            seg = np.zeros((CT, D), np.float32)
            seg[:ct] = xr_full[t0 : t0 + ct]
            xr_blk[:, t] = seg.reshape(NCH, P, D).transpose(1, 0, 2)
            wseg = np.zeros(CT, np.float32)
            wseg[:ct] = wv_full[t0 : t0 + ct]
            wv_blk[:, t * NCH : (t + 1) * NCH] = wseg.reshape(NCH, P).T
        im = {
            "xT": xT_blk,
            "xres": xr_blk,
            "w1": _fp8(_wlay(w1[e], KO1, MO1), SW),
            "w2": _fp8(
                np.asarray(w2)[e]
                .reshape(KO2 // 2, 2, P, 2, 512)
                .transpose(2, 3, 0, 1, 4),
                SW,
            ),
            "b1": np.ascontiguousarray(np.asarray(b1)[e].reshape(MO1, P).T),
            "wv": wv_blk,
        }
        if use_lng:
            im["ln_g"] = np.broadcast_to(np.asarray(ln_g)[e], (P, D)).copy()
        if use_lnb:
            im["ln_b"] = np.broadcast_to(np.asarray(ln_b)[e], (P, D)).copy()
        in_maps.append(im)

    def _run_once():
        return run_bass_kernel_spmd(nc, in_maps, core_ids=list(range(E)))

    # first execution after a fresh compile has been observed to fail
    # transiently (device-side) — either raising or returning NaN-poisoned
    # buffers; one retry has always succeeded
    import time

    try:
        res = _run_once()
    except Exception:
        time.sleep(2)
        res = _run_once()
    if any(
        np.isnan(np.asarray(res.results[e]["out"], np.float32)).any() for e in range(E)
    ):
        time.sleep(2)
        res = _run_once()
    kernel.last_results = res

    # combine: token t's two contributions live at known (expert, slot) pairs
    slot0 = np.empty(T, np.int64)
    slot1 = np.empty(T, np.int64)
    for e in range(E):
        n0 = int(np.sum(e0 == e))
        slot0[e0 == e] = np.arange(n0)
        slot1[e1 == e] = n0 + np.arange(int(np.sum(e1 == e)))
    Y = np.stack(
        [np.asarray(res.results[e]["out"], np.float32) for e in range(E)]
    )  # [E, C, D]
    out = Y[e0, slot0] + Y[e1, slot1]  # [T, D]
    return out.reshape(x.shape).astype(np.float32)


# revision 58
# speedup vs baseline: 1.0463x; 1.0463x over previous
"""MoE (E=8 experts, top-2, D=1024, T=8192) — expert-parallel Trainium2 kernel.

Strategy (per the expert-parallel sharding hint):
  - Host computes the gate (0.1% of FLOPs: scores, top-2, softmax) and uses it
    to shard tokens: each of the 8 NeuronCores owns one expert and receives
    exactly the tokens routed to it (padded to a common capacity C).
  - Each core runs the dense expert FFN + layernorm + combine-weight scaling
    over its routed tokens: 99.9% of the FLOPs.
  - Host gathers the per-expert outputs back into token order (pure gather —
    slot assignment makes a scatter unnecessary) and sums the K=2 contributions.

Device dataflow — mm1 weights-stationary, mm2 TOKENS-stationary:
  mm1: stationary w1 chunks, moving xT [d, tok] -> psum hT [2D-feat, tok]
       -> gelu -> hT fp8 (features on partitions).
  mm2: stationary hT chunks [f=256(DR), tok=128], moving w2 [f, d] -> psum
       z [tok 128, d 512] x2 banks — TOKENS ON PARTITIONS.
  With tokens on partitions the whole LN epilogue runs on free-axis
  primitives and the PE does ZERO stats work (the previous design burned
  ~20% of PE time on ones@z / ones@z^2 stats matmuls):
    z = psum + xres            (DVE tensor_tensor, f32)
    mean/var                   (DVE bn_stats / bn_aggr, exact f32)
    rstd = rsqrt(var+eps)      (DVE-only bit-trick seed + 2 Newton steps —
                                keeps Sqrt off ACT so the gelu activation
                                table is never thrashed)
    out = (z - mu)*rstd*wv     (ONE fused DVE tensor_scalar per 512-bank,
                                per-partition scalar operands)
  and the output DMA writes token-major [C, D] rows (fat 4KB descriptors,
  pure row-gather combine on host).

The FFN matmuls run in fp8-e4m3 with perf_mode=DoubleRow (measured peak:
512-col DR matmul = 215ns = 157 TF/s). Weights are host-prescaled by 2^10
(and x by 2^4 for mm1) so the e4m3 lattice is used well clear of denormals;
the gelu activation unscales mm1 by 2^-14. mm2's 2^10 output scale is never
unscaled on the z path: layernorm is scale-invariant (eps is scaled by 2^20,
rstd comes out 2^-10-scaled and cancels it), so the device output is the
exact unscaled wv-weighted LN. w1/w2 are fully SBUF-resident in fp8 (4 MB),
host-relaid so every DMA lands as one fat contiguous descriptor per
partition. mm1 is software-pipelined one tile ahead of mm2, with mm2
accumulation groups interleaved between mm1 groups so the in-order PE always
has an independent group queued behind any group waiting on DMA or the DVE.
"""

import sys

sys.path.insert(0, "/opt/trn_rl_repo")

import ml_dtypes
import numpy as np

E, K, D = 8, 2, 1024
H = 2 * D  # ffn hidden
B, S = 2, 4096
T = B * S
CT = 512  # max token tile (PSUM bank = 512 f32)
CT_MIN = 32  # remainder-tile granularity
P = 128
NCH = CT // P  # 4 token chunks per tile
KO1 = D // P  # 8  k-chunks for mm1 (4 DoubleRow pairs)
MO1 = H // P  # 16 m-chunks for mm1
KO2 = H // P  # 16 k-chunks for mm2 (8 DoubleRow pairs)

SW = 1024.0  # weight prescale (2^10)
SX = 16.0  # x prescale for the fp8 mm1 moving operand (2^4)
FP8_MAX = 240.0  # TRN float8e4 max normal
MAGICF = float(0x5F3759DF)  # rsqrt seed magic (bit trick in float arithmetic)

_kernel_cache = {}


def _tiles_for(C):
    tiles = [(i * CT, CT) for i in range(C // CT)]
    if C % CT:
        tiles.append((C - C % CT, C % CT))
    return tiles


def _cws_for(ct):
    """Token-chunk widths of a tile: e.g. 512 -> [128,128,128,128]; 160 -> [128,32]."""
    return [min(P, ct - c * P) for c in range((ct + P - 1) // P)]


def _build_bass(C, use_lng, use_lnb):
    """Build the per-core expert-FFN kernel for capacity C (multiple of CT_MIN)."""
    import concourse.tile as tile
    from concourse import bacc, mybir

    f32 = mybir.dt.float32
    i32 = mybir.dt.int32
    fp8 = mybir.dt.float8e4
    AF = mybir.ActivationFunctionType
    OP = mybir.AluOpType
    DR = mybir.MatmulPerfMode.DoubleRow

    assert C % CT_MIN == 0
    tiles = _tiles_for(C)
    NT = len(tiles)

    nc = bacc.Bacc("TRN2", target_bir_lowering=False, debug=False)
    # xT/xres ship in per-tile-padded block layouts so each tile's DMA is one
    # contiguous descriptor per partition
    xT_d = nc.dram_tensor("xT", [P, NT, KO1, CT], fp8, kind="ExternalInput")
    xres_d = nc.dram_tensor("xres", [P, NT, NCH, D], f32, kind="ExternalInput")
    w1_d = nc.dram_tensor("w1", [P, MO1, KO1, P], fp8, kind="ExternalInput")
    # w2 moving layout, bank-major: [p, bank, k, j, d'] = w2[(2k+j)*128+p,
    # bank*512+d'] (DR pair on j). Bank-major lets mm2 run all bank-0 groups
    # before any bank-1 group, so the startup only races the FIRST 1MB of w2.
    w2_d = nc.dram_tensor("w2", [P, 2, KO2 // 2, 2, 512], fp8, kind="ExternalInput")
    b1_d = nc.dram_tensor("b1", [P, MO1], f32, kind="ExternalInput")
    wv_d = nc.dram_tensor("wv", [P, NT * NCH], f32, kind="ExternalInput")
    if use_lng:
        lng_d = nc.dram_tensor("ln_g", [P, D], f32, kind="ExternalInput")
    if use_lnb:
        lnb_d = nc.dram_tensor("ln_b", [P, D], f32, kind="ExternalInput")
    bf16 = mybir.dt.bfloat16
    out_d = nc.dram_tensor("out", [C, D], bf16, kind="ExternalOutput")

    with tile.TileContext(nc) as tc:
        with (
            tc.tile_pool(name="singles", bufs=1) as singles,
            tc.tile_pool(name="xp", bufs=3) as xp,
            tc.tile_pool(name="hp", bufs=2) as hp,
            tc.tile_pool(name="zp", bufs=2) as zp,
            tc.tile_pool(name="xrp", bufs=2) as xrp,
            tc.tile_pool(name="stp", bufs=2) as stp,
            tc.tile_pool(name="psmm", bufs=5, space="PSUM") as psmm,
            tc.tile_pool(name="psm2", bufs=3, space="PSUM") as psm2,
        ):
            # ---- resident data ----
            # Startup is HWDGE-ramp-bound (~160 GB/s for the first few us), so
            # the first matmul's dependency set is kept minimal: tile-0 xT is
            # DMAd in two k-halves (one per ring) and w1 chunk 0 in two
            # m-halves, so subtile dep-tracking releases mm1 group 0 after
            # ~384KB instead of ~2.5MB. b1/wv are tiny and go first. The
            # warm-up gelu pulls the ACT_TABLE_LOAD (1.3us) into the DMA wait
            # instead of stalling the first real gelu (and the psum ring).
            # Every dma_start costs ~630ns of ISSUE time on its sequencer and
            # all HWDGE traffic fans out to the same 16 DMA engines, so ring
            # choice is about sequencer headroom, not bandwidth: the whole
            # startup weight stream issues from the (otherwise idle) SP
            # queue, keeping the ACT queue free to start gelu 0 the moment
            # mm1 group 0 closes.
            warm = singles.tile([P, 1], f32)
            nc.vector.memset(warm[:], 0.0)
            nc.scalar.activation(out=warm[:], in_=warm[:], func=AF.Gelu, bias=0.0, scale=0.0)
            xT_tiles = [xp.tile([P, KO1, CT], fp8, name="xT_sb") for _ in range(3)]
            b1_sb = singles.tile([P, MO1], f32)
            wv_sb = singles.tile([P, NT * NCH], f32)
            # w1 lives in 4 tiles grouped by consumption time: m0/m1 as two
            # tiny transfers (first-matmul deps), then three bulk transfers.
            # Each DMA targets a whole tile, so dependency release needs no
            # subtile tracking, and the issue count (5 + xT halves) keeps
            # both sequencers' ~630ns/issue overhead off the critical path.
            w1_groups = [[0, 1], [2, 3], [4, 5], [6, 7, 8, 9], [10, 11, 12, 13, 14, 15]]
            w1_tiles = [
                singles.tile([P, len(ms), KO1, P], fp8, name=f"w1g{i}")
                for i, ms in enumerate(w1_groups)
            ]
            w1_map = {}
            for gi, ms in enumerate(w1_groups):
                for li, m in enumerate(ms):
                    w1_map[m] = (gi, li)
            nc.sync.dma_start(xT_tiles[0][:, 0 : KO1 // 2], xT_d[:, 0, 0 : KO1 // 2])
            nc.scalar.dma_start(b1_sb[:], b1_d[:])
            nc.sync.dma_start(w1_tiles[0][:, 0:1], w1_d[:, 0:1])
            nc.scalar.dma_start(xT_tiles[0][:, KO1 // 2 : KO1], xT_d[:, 0, KO1 // 2 : KO1])
            nc.scalar.dma_start(w1_tiles[0][:, 1:2], w1_d[:, 1:2])
            nc.sync.dma_start(w1_tiles[1][:], w1_d[:, 2:4])
            nc.scalar.dma_start(w1_tiles[2][:], w1_d[:, 4:6])
            nc.scalar.dma_start(w1_tiles[3][:], w1_d[:, 6:10])
            nc.sync.dma_start(w1_tiles[4][:], w1_d[:, 10:16])
            nc.scalar.dma_start(wv_sb[:], wv_d[:])
            # PE warm-up: dep-free dummy matmuls run during the startup DMA
            # wait, ramping the tensor engine's DVFS clock to max so the
            # first real matmuls start at full speed instead of re-ramping
            wsrc = singles.tile([P, 2, 512], fp8)
            nc.gpsimd.memset(wsrc[:], 1.0)
            wps = psmm.tile([P, CT], f32, name="ps_mm")
            for _ in range(20):
                nc.tensor.matmul(
                    wps[:],
                    wsrc[:, :, :128],
                    wsrc[:],
                    start=True,
                    stop=True,
                    perf_mode=DR,
                )
            if use_lng:
                lng_sb = singles.tile([P, D], f32)
                nc.scalar.dma_start(lng_sb[:], lng_d[:])
            if use_lnb:
                lnb_sb = singles.tile([P, D], f32)
                nc.scalar.dma_start(lnb_sb[:], lnb_d[:])
            w2_ch = [singles.tile([P, KO2 // 2, 2, 512], fp8, name=f"w2c{i}") for i in range(2)]

            def emit_w2_chunk(i):
                nc.scalar.dma_start(w2_ch[i][:], w2_d[:, i])

            def new_state(t, t0, ct):
                """Allocated ahead of mm1(t); the xres DMA is emitted inside
                emit_mm1(t) behind a few of tile t's gelus, so it lands a full
                tile before mm2(t) consumes it but never steals HBM bandwidth
                from the startup weight stream."""
                return {
                    "t": t, "t0": t0, "ct": ct, "cws": _cws_for(ct),
                    "hT_sb": None, "norm_prev": None,
                    "xres": xrp.tile([P, NCH, D], f32, name="xres"),
                    "z": zp.tile([P, NCH, 2, 512], f32, name="z_sb"),
                    "occ": zp.tile([P, NCH, D], bf16, name="occ"),
                    "bst": stp.tile([P, NCH, 2, 6], f32, name="bst"),
                    "mv": stp.tile([P, NCH * 2], f32, name="mv"),
                    "rw": stp.tile([P, NCH], f32, name="rw"),
                    "nb": stp.tile([P, NCH], f32, name="nb"),
                }

            def emit_mm1(t, st, prev_st):
                """mm1 (fp8 DoubleRow) + gelu into a fresh hT tile; mm2 groups
                of the previous tile interleave so the PE always has an
                independent accumulation group queued behind a stalled one."""
                ct = st["ct"]
                xT_sb = xT_tiles[t % 3]
                hT_sb = hp.tile([P, KO2, CT], fp8, name="hT_sb")
                for m in range(MO1):
                    gi, li = w1_map[m]
                    ps = psmm.tile([P, CT], f32, name="ps_mm")
                    for k in range(KO1 // 2):
                        nc.tensor.matmul(
                            ps[:, :ct],
                            w1_tiles[gi][:, li, 2 * k : 2 * k + 2, :],
                            xT_sb[:, 2 * k : 2 * k + 2, :ct],
                            start=(k == 0),
                            stop=(k == KO1 // 2 - 1),
                            perf_mode=DR,
                        )
                    # gelu(2^-14 * ps + b1) -> fp8 h (unscaled)
                    nc.scalar.activation(
                        out=hT_sb[:, m, :ct],
                        in_=ps[:, :ct],
                        func=AF.Gelu,
                        bias=b1_sb[:, m : m + 1],
                        scale=1.0 / (SW * SX),
                    )
                    if t == 0 and m in (3, 6):
                        emit_w2_chunk((m - 3) // 3)
                    if m == (6 if t == 0 else 2):
                        nc.scalar.dma_start(st["xres"][:], xres_d[:, t])
                    # interleave offset 5 (not 1): tile 0's bank-0 z-adds
                    # otherwise outrun the xres(0) DMA, pinning psum banks
                    # and stalling the PE into window 1; the offset trades
                    # pure-mm1 slots at the window head for xres slack
                    if prev_st is not None and 5 <= m <= 12:
                        mm2_group(prev_st, m - 5)
                st["hT_sb"] = hT_sb

            def mm2_group(st, slot, cb=None):
                """One (token-chunk, d-bank) accumulation group: psum[tok, d]
                += hT.T @ w2 over all 16 k-chunks, then the z/stats chain.
                The previous tile's (ACT-engine) normalize rides one chunk at
                a time between this tile's gelus. Main-loop order is
                bank-major (all bank-0 groups first) so the first half of w2
                isn't needed until 4 groups later than the first."""
                if st["norm_prev"] is not None:
                    norm_chunk(st["norm_prev"], slot)
                b, c = divmod(slot, NCH) if cb is None else cb
                if c >= len(st["cws"]):
                    return
                cw = st["cws"][c]
                ps = psm2.tile([P, 512], f32, name="ps2")
                for k in range(KO2 // 2):
                    nc.tensor.matmul(
                        ps[:cw, :],
                        st["hT_sb"][:, 2 * k : 2 * k + 2, c * P : c * P + cw],
                        w2_ch[b][:, k],
                        start=(k == 0),
                        stop=(k == KO2 // 2 - 1),
                        perf_mode=DR,
                    )
                zc = st["z"][:cw, c, b]
                nc.vector.tensor_tensor(
                    zc, ps[:cw, :], st["xres"][:cw, c, b * 512 : (b + 1) * 512], OP.add
                )
                nc.vector.bn_stats(st["bst"][:cw, c, b], zc)
                if b == 1:
                    nc.vector.bn_aggr(st["mv"][:cw, 2 * c : 2 * c + 2], st["bst"][:cw, c])

            def stats_range(st, lo, hi, newton=2):
                """Batched rstd for chunks [lo, hi): rw[:, c] = wv * rsqrt(
                var_c + eps), computed entirely on the DVE (bit-trick seed +
                2 Newton steps) so the ACT gelu table is never thrashed by a
                Sqrt."""
                t = st["t"]
                mv, rw = st["mv"], st["rw"]
                vb = stp.tile([P, NCH], f32, name="vb")
                # var is 2^20-scaled; eps scales to match and rstd comes out
                # 2^-10-scaled, cancelling z's 2^10 scale in the normalize
                nc.vector.tensor_scalar(
                    vb[:, lo:hi], mv[:, 2 * lo + 1 : 2 * hi : 2], 1.0, 1e-6 * SW * SW,
                    OP.mult, OP.add,
                )
                sd = stp.tile([P, NCH], f32, name="sd")
                nc.vector.tensor_copy(sd[:, lo:hi], vb[:, lo:hi].bitcast(i32))
                nc.vector.tensor_scalar(sd[:, lo:hi], sd[:, lo:hi], -0.5, MAGICF, OP.mult, OP.add)
                y0 = stp.tile([P, NCH], i32, name="y0")
                nc.vector.tensor_copy(y0[:, lo:hi], sd[:, lo:hi])
                yf = y0[:, lo:hi].bitcast(f32)
                a_sb = stp.tile([P, NCH], f32, name="a_sb")
                for _ in range(newton):
                    nc.vector.tensor_tensor(a_sb[:, lo:hi], vb[:, lo:hi], yf, OP.mult)
                    nc.vector.tensor_tensor(a_sb[:, lo:hi], a_sb[:, lo:hi], yf, OP.mult)
                    nc.vector.tensor_scalar(a_sb[:, lo:hi], a_sb[:, lo:hi], -0.5, 1.5, OP.mult, OP.add)
                    nc.vector.tensor_tensor(yf, yf, a_sb[:, lo:hi], OP.mult)
                nc.vector.tensor_tensor(
                    rw[:, lo:hi], yf, wv_sb[:, t * NCH + lo : t * NCH + hi], OP.mult
                )
                # normalize runs on ACT as out = Identity(z*rw + nb) with
                # per-partition scalars, so nb = -mu*rw
                nb = st["nb"]
                nc.vector.tensor_tensor(
                    nb[:, lo:hi], mv[:, 2 * lo : 2 * hi : 2], rw[:, lo:hi], OP.mult
                )
                nc.vector.tensor_scalar(nb[:, lo:hi], nb[:, lo:hi], -1.0, None, OP.mult)

            def norm_chunk(st, g, dve=False):
                """occ = Identity(z*rw - mu*rw) on the ACT engine (Identity
                lives in the gelu table: no table reload, and the DVE stays
                free for the z/bn chain); bank 1 also DMAs the finished
                [cw, D] token rows out (fat row descriptors). dve=True runs
                the same affine on the DVE instead — used for the final
                chunk so its two banks normalize on two engines in parallel."""
                c, b = divmod(g, 2)
                if c >= len(st["cws"]):
                    return
                cw = st["cws"][c]
                zc = st["z"][:cw, c, b]
                oc = st["occ"][:cw, c, b * 512 : (b + 1) * 512]
                if dve:
                    nc.vector.tensor_scalar(
                        oc, zc, st["mv"][:cw, 2 * c : 2 * c + 1],
                        st["rw"][:cw, c : c + 1], OP.subtract, OP.mult,
                    )
                else:
                    nc.scalar.activation(
                        out=oc, in_=zc, func=AF.Identity,
                        bias=st["nb"][:cw, c : c + 1], scale=st["rw"][:cw, c : c + 1],
                    )
                if use_lng:
                    nc.vector.tensor_tensor(
                        oc, oc, lng_sb[:cw, b * 512 : (b + 1) * 512], OP.mult
                    )
                if use_lnb:
                    lb = stp.tile([P, 512], f32, name="lb")
                    nc.vector.tensor_scalar(
                        lb[:cw, :], lnb_sb[:cw, b * 512 : (b + 1) * 512],
                        wv_sb[:cw, st["t"] * NCH + c : st["t"] * NCH + c + 1],
                        None, OP.mult,
                    )
                    nc.vector.tensor_tensor(oc, oc, lb[:cw, :], OP.add)
                if b == 1:
                    r0 = st["t0"] + c * P
                    nc.sync.dma_start(out_d[r0 : r0 + cw, :], st["occ"][:cw, c])

            # xT(t) is consumed only by mm1(t) (residual uses the xres
            # stream); prefetches are queued after the w1 chunks so they never
            # delay the critical w1 stream. stats+normalize+out-DMA of tile t
            # are emitted in window t+1 (right behind mm2(t)'s z chain), so
            # the out DMAs spread across the kernel instead of bunching in
            # the tail; the DVE has ~12us/window of slack for them.
            prev_st = None
            for t, (t0, ct) in enumerate(tiles):
                st = new_state(t, t0, ct)
                emit_mm1(t, st, prev_st)
                for tn_i in ([1, 2] if t == 0 else [t + 2]):
                    if tn_i < NT:
                        nc.sync.dma_start(xT_tiles[tn_i % 3][:], xT_d[:, tn_i])
                if prev_st is not None:
                    stats_range(prev_st, 0, len(prev_st["cws"]))
                    st["norm_prev"] = prev_st
                prev_st = st
            # epilogue: the last tile drains chunk-at-a-time (mm2 -> stats ->
            # norm -> DMA) so each chunk's serial stats/norm chain overlaps
            # the next chunk's matmuls and the tail is one chunk deep; norm
            # of tile NT-2 rides the epilogue mm2 groups as usual
            nch_l = len(prev_st["cws"])
            # a short last tile has fewer mm2 groups than the previous tile
            # has norm chunks — flush the rides the loop below won't reach
            if prev_st["norm_prev"] is not None:
                for g in range(2 * nch_l, 2 * NCH):
                    norm_chunk(prev_st["norm_prev"], g)
            for c in range(nch_l):
                mm2_group(prev_st, 2 * c, cb=(0, c))
                mm2_group(prev_st, 2 * c + 1, cb=(1, c))
                # the final chunk's serial stats chain IS the kernel tail;
                # it is ~all padding slots (only the max-load expert has a
                # handful of real tokens there), so one Newton step suffices
                last = c == nch_l - 1
                stats_range(prev_st, c, c + 1, newton=1 if last else 2)
                norm_chunk(prev_st, 2 * c, dve=last)
                norm_chunk(prev_st, 2 * c + 1)

    nc.finalize()
    return nc


def _route(x, gate_w):
    """Host gate: top-2 per token + softmax combine weights (matches
    jax.lax.top_k tie-breaking: lower index wins)."""
    xt = x.reshape(-1, D)
    scores = xt.astype(np.float32) @ gate_w.astype(np.float32)  # [T, E]
    e0 = np.argmax(scores, axis=1)
    s0 = scores[np.arange(T), e0]
    masked = scores.copy()
    masked[np.arange(T), e0] = -np.inf
    e1 = np.argmax(masked, axis=1)
    s1 = masked[np.arange(T), e1]
    # softmax over the two selected scores
    mx = np.maximum(s0, s1)
    z0 = np.exp((s0 - mx).astype(np.float64))
    z1 = np.exp((s1 - mx).astype(np.float64))
    den = z0 + z1
    w0 = (z0 / den).astype(np.float32)
    w1 = (z1 / den).astype(np.float32)
    return xt, e0, e1, w0, w1


def _fp8(a, scale):
    return np.clip(a * scale, -FP8_MAX, FP8_MAX).astype(ml_dtypes.float8_e4m3)


def _wlay(w, ko, mo):
    """[ko*P, mo*P] -> [P, mo, ko, P]: partition-contiguous DMA layout that
    matches the lhsT access pattern w_sb[:, m, 2k:2k+2, :]."""
    return np.ascontiguousarray(
        np.asarray(w).reshape(ko, P, mo, P).transpose(1, 2, 0, 3)
    )


def kernel(x, gate_w, w1, b1, w2, b2, ln_g, ln_b):
    from concourse.bass_utils import run_bass_kernel_spmd

    x = np.asarray(x)
    xt, e0, e1, wk0, wk1 = _route(x, np.asarray(gate_w))

    # slot assignment: expert e's token list = tokens with e0==e, then e1==e
    idx_e, wv_e = [], []
    for e in range(E):
        i0 = np.nonzero(e0 == e)[0]
        i1 = np.nonzero(e1 == e)[0]
        idx_e.append(np.concatenate([i0, i1]))
        wv_e.append(np.concatenate([wk0[i0], wk1[i1]]))
    maxn = max(len(i) for i in idx_e)
    C = max(CT_MIN, -(-maxn // CT_MIN) * CT_MIN)
    tiles = _tiles_for(C)
    NT = len(tiles)

    b2 = np.asarray(b2, np.float32)
    use_lng = bool(np.any(np.asarray(ln_g) != 1))
    use_lnb = bool(np.any(np.asarray(ln_b) != 0))
    key = (C, use_lng, use_lnb)
    if key not in _kernel_cache:
        _kernel_cache[key] = _build_bass(C, use_lng, use_lnb)
    nc = _kernel_cache[key]

    in_maps = []
    for e in range(E):
        n = len(idx_e[e])
        xTe = np.zeros((D, C), np.float32)
        xTe[:, :n] = xt[idx_e[e]].T
        # mm1 moving operand: per-tile padded fp8 block layout [P, NT, KO1, CT]
        x8 = _fp8(xTe, SX).reshape(KO1, P, C)
        xT_blk = np.zeros((P, NT, KO1, CT), ml_dtypes.float8_e4m3)
        # residual stream, token-major rows (b2 folded in), 2^10-scaled
        xr_full = np.zeros((C, D), np.float32)
        xr_full[:n] = (xt[idx_e[e]] + b2[e]) * SW
        wv_full = np.zeros(C, np.float32)
        wv_full[:n] = wv_e[e]
        xr_blk = np.zeros((P, NT, NCH, D), np.float32)
        wv_blk = np.zeros((P, NT * NCH), np.float32)
        for t, (t0, ct) in enumerate(tiles):
            xT_blk[:, t, :, :ct] = x8[:, :, t0 : t0 + ct].transpose(1, 0, 2)
            seg = np.zeros((CT, D), np.float32)
            seg[:ct] = xr_full[t0 : t0 + ct]
            xr_blk[:, t] = seg.reshape(NCH, P, D).transpose(1, 0, 2)
            wseg = np.zeros(CT, np.float32)
            wseg[:ct] = wv_full[t0 : t0 + ct]
            wv_blk[:, t * NCH : (t + 1) * NCH] = wseg.reshape(NCH, P).T
        im = {
            "xT": xT_blk,
            "xres": xr_blk,
            "w1": _fp8(_wlay(w1[e], KO1, MO1), SW),
            "w2": _fp8(
                np.asarray(w2)[e]
                .reshape(KO2 // 2, 2, P, 2, 512)
                .transpose(2, 3, 0, 1, 4),
                SW,
            ),
            "b1": np.ascontiguousarray(np.asarray(b1)[e].reshape(MO1, P).T),
            "wv": wv_blk,
        }
        if use_lng:
            im["ln_g"] = np.broadcast_to(np.asarray(ln_g)[e], (P, D)).copy()
        if use_lnb:
            im["ln_b"] = np.broadcast_to(np.asarray(ln_b)[e], (P, D)).copy()
        in_maps.append(im)

    def _run_once():
        return run_bass_kernel_spmd(nc, in_maps, core_ids=list(range(E)))

    # first execution after a fresh compile has been observed to fail
    # transiently (device-side) — either raising or returning NaN-poisoned
    # buffers; one retry has always succeeded
    import time

    try:
        res = _run_once()
    except Exception:
        time.sleep(2)
        res = _run_once()
    if any(
        np.isnan(np.asarray(res.results[e]["out"], np.float32)).any() for e in range(E)
    ):
        time.sleep(2)
        res = _run_once()
    kernel.last_results = res

    # combine: token t's two contributions live at known (expert, slot) pairs
    slot0 = np.empty(T, np.int64)
    slot1 = np.empty(T, np.int64)
    for e in range(E):
        n0 = int(np.sum(e0 == e))
        slot0[e0 == e] = np.arange(n0)
        slot1[e1 == e] = n0 + np.arange(int(np.sum(e1 == e)))
    Y = np.stack(
        [np.asarray(res.results[e]["out"], np.float32) for e in range(E)]
    )  # [E, C, D]
    out = Y[e0, slot0] + Y[e1, slot1]  # [T, D]
    return out.reshape(x.shape).astype(np.float32)
